# revision 12
# baseline (speedup 1.0000x reference)
"""DAM encoder Trainium2 kernel.

Math (per batch item, identical to the reference up to fp rounding):
  a_e = relu(a @ Wp + bp); b_e likewise                  [L, H]
  Fa  = relu(a_e @ Wf + bf); Fb likewise                 (masks on Fa/Fb fold out)
  att = Fa @ Fb^T                                        [L, L]
  E   = exp(att) * mask-bias (softmax without row-max: values bounded ~e^30)
  soft1 = E / (rowsum_j E + eps); soft2 = E^T / (rowsum_i E^T + eps)
  beta = soft1 @ b_e; alpha = soft2 @ a_e
  v1 = relu([a_e, beta] @ Wg + bg) * am; v2 likewise
  out = [v1.sum(L), v2.sum(L), v1.max(L), v2.max(L)]     [4H]

Layouts on chip (partition dim first):
  xT     [Dp=304, L]  (host pre-transposed, row 300 = ones => bias via matmul)
  aeT    [H, L]   (for F/G matmuls)      ae [L, H] (for alpha matmul lhsT)
  faT/fbT[H, L]
  et chunks [128 of Lb, La] = exp(attT)+bm-bias ; e chunks [128 of La, Lb]
  s1 = ones^T @ et-chunks  -> [128(bcast), La] rows all equal rowsum
  betaT [H, La] = (b_e^T-as-lhsT @ et) * R1 ; alphaT likewise
  v1T   [H, La] -> masked reduce along free dim.

Data-parallel over batch: 16 items -> 8 cores x 2 items.
"""

import os
import numpy as np

import concourse.bass as bass
import concourse.bacc as bacc
import concourse.mybir as mybir
import concourse.tile as tile
from concourse.bass_utils import run_bass_kernel_spmd

B, L, D, H = 16, 1024, 300, 256
DP = 304            # 300 data rows + 1 ones row + 3 zero pad
NCORES = 8
IPC = B // NCORES   # items per core
PK = [128, 128, 48]  # partition chunking of DP

F32 = mybir.dt.float32
F32R = mybir.dt.float32r
AF = mybir.ActivationFunctionType
OP = mybir.AluOpType
AX = mybir.AxisListType.X

MASK_BIAS = -100.0  # exp(att + MASK_BIAS) == 0 relative to unmasked terms


def _round_fp32r(x):
    """Round fp32 to the fp32r format: 11 mantissa bits, low 12 bits zero (RNE)."""
    u = np.ascontiguousarray(x, np.float32).view(np.uint32)
    r = (u + 0x7FF + ((u >> 12) & 1)) & np.uint32(0xFFFFF000)
    return r.view(np.float32)


def _build():
    nc = bacc.Bacc("TRN2", target_bir_lowering=False, debug=False)
    xa = nc.dram_tensor("xa", [IPC, DP, L], F32R, kind="ExternalInput")
    xb = nc.dram_tensor("xb", [IPC, DP, L], F32R, kind="ExternalInput")
    wp = nc.dram_tensor("wp", [DP, H], F32R, kind="ExternalInput")
    wf = nc.dram_tensor("wf", [H, H], F32R, kind="ExternalInput")
    wg = nc.dram_tensor("wg", [2 * H, H], F32R, kind="ExternalInput")
    bfc = nc.dram_tensor("bfc", [128, 2], F32, kind="ExternalInput")
    bgc = nc.dram_tensor("bgc", [128, 2], F32, kind="ExternalInput")
    # (mask-1)*100 per chunk column: exp bias
    amb = nc.dram_tensor("amb", [IPC, 128, 8], F32, kind="ExternalInput")
    bmb = nc.dram_tensor("bmb", [IPC, 128, 8], F32, kind="ExternalInput")
    amf = nc.dram_tensor("amf", [IPC, L], F32, kind="ExternalInput")
    bmf = nc.dram_tensor("bmf", [IPC, L], F32, kind="ExternalInput")
    onesd = nc.dram_tensor("onesd", [128, 128], F32R, kind="ExternalInput")
    out = nc.dram_tensor("out", [IPC, 128, 8], F32, kind="ExternalOutput")

    with tile.TileContext(nc) as tc, \
            tc.tile_pool(name="consts", bufs=1) as consts, \
            tc.tile_pool(name="io", bufs=2) as io, \
            tc.tile_pool(name="acts", bufs=1) as acts, \
            tc.tile_pool(name="ech", bufs=3) as ech, \
            tc.tile_pool(name="pp", bufs=8, space="PSUM") as pp:

        # ---------------- constants ----------------
        wp_sb = consts.tile([128, 3, H], F32R, name="wp_sb")
        for k in range(3):
            nc.gpsimd.dma_start(out=wp_sb[:PK[k], k, :], in_=wp[k * 128:k * 128 + PK[k], :])
        wf_sb = consts.tile([128, 2, H], F32R, name="wf_sb")
        for k in range(2):
            nc.gpsimd.dma_start(out=wf_sb[:, k, :], in_=wf[k * 128:(k + 1) * 128, :])
        wg_sb = consts.tile([128, 4, H], F32R, name="wg_sb")
        for k in range(4):
            nc.gpsimd.dma_start(out=wg_sb[:, k, :], in_=wg[k * 128:(k + 1) * 128, :])
        bf_sb = consts.tile([128, 2], F32, name="bf_sb")
        nc.gpsimd.dma_start(out=bf_sb[:, :], in_=bfc[:, :])
        bg_sb = consts.tile([128, 2], F32, name="bg_sb")
        nc.gpsimd.dma_start(out=bg_sb[:, :], in_=bgc[:, :])
        ones_sb = consts.tile([128, 128], F32R, name="ones_sb")
        nc.gpsimd.dma_start(out=ones_sb[:, :], in_=onesd[:, :])

        for it in range(IPC):
            # ---------------- per-item loads ----------------
            xa_sb = io.tile([128, 3, L], F32R, name="xa_sb", tag="xa")
            xb_sb = io.tile([128, 3, L], F32R, name="xb_sb", tag="xb")
            for k in range(3):
                nc.gpsimd.dma_start(out=xa_sb[:PK[k], k, :], in_=xa[it, k * 128:k * 128 + PK[k], :])
                nc.gpsimd.dma_start(out=xb_sb[:PK[k], k, :], in_=xb[it, k * 128:k * 128 + PK[k], :])
            amb_sb = io.tile([128, 8], F32, name="amb_sb", tag="amb")
            bmb_sb = io.tile([128, 8], F32, name="bmb_sb", tag="bmb")
            nc.gpsimd.dma_start(out=amb_sb[:, :], in_=amb[it])
            nc.gpsimd.dma_start(out=bmb_sb[:, :], in_=bmb[it])
            AM_sb = io.tile([128, L], F32, name="AM_sb", tag="AM")
            BM_sb = io.tile([128, L], F32, name="BM_sb", tag="BM")
            nc.gpsimd.dma_start(
                out=AM_sb[:, :], in_=bass.AP(tensor=amf, offset=it * L, ap=[[0, 128], [1, L]]))
            nc.gpsimd.dma_start(
                out=BM_sb[:, :], in_=bass.AP(tensor=bmf, offset=it * L, ap=[[0, 128], [1, L]]))

            res = io.tile([128, 8], F32, name="res", tag="res")

            def _finish_early(srcap):
                for c in range(8):
                    nc.vector.reduce_sum(out=res[:, c:c + 1], in_=srcap, axis=AX)
                nc.gpsimd.dma_start(out=out[it], in_=res[:, :])

            # ---------------- projection ----------------
            aeT = acts.tile([128, 2, L], F32R, name="aeT", tag="aeT")
            beT = acts.tile([128, 2, L], F32R, name="beT", tag="beT")
            ae = acts.tile([128, 8, H], F32R, name="ae", tag="ae")
            be = acts.tile([128, 8, H], F32R, name="be", tag="be")
            for dst, src in ((aeT, xa_sb), (beT, xb_sb)):
                for m in range(2):
                    for n in range(2):
                        ps = pp.tile([128, 512], F32, name="ps", tag="ps")
                        for k in range(3):
                            nc.tensor.matmul(
                                ps[:, :], wp_sb[:PK[k], k, m * 128:(m + 1) * 128],
                                src[:PK[k], k, n * 512:(n + 1) * 512],
                                start=(k == 0), stop=(k == 2))
                        nc.vector.tensor_scalar_max(
                            out=dst[:, m, n * 512:(n + 1) * 512], in0=ps[:, :], scalar1=0.0)
            for dst, src in ((ae, xa_sb), (be, xb_sb)):
                for m in range(8):
                    ps = pp.tile([128, 512], F32, name="ps", tag="ps")
                    for k in range(3):
                        nc.tensor.matmul(
                            ps[:, :H], src[:PK[k], k, m * 128:(m + 1) * 128],
                            wp_sb[:PK[k], k, :], start=(k == 0), stop=(k == 2))
                    nc.vector.tensor_scalar_max(out=dst[:, m, :], in0=ps[:, :H], scalar1=0.0)

            if int(os.environ.get("KBISECT", "9")) <= 1:
                _finish_early(aeT[:, 0, :])
                continue

            # ---------------- F ----------------
            faT = acts.tile([128, 2, L], F32R, name="faT", tag="faT")
            fbT = acts.tile([128, 2, L], F32R, name="fbT", tag="fbT")
            for dst, src in ((faT, aeT), (fbT, beT)):
                for m in range(2):
                    for n in range(2):
                        ps = pp.tile([128, 512], F32, name="ps", tag="ps")
                        for k in range(2):
                            nc.tensor.matmul(
                                ps[:, :], wf_sb[:, k, m * 128:(m + 1) * 128],
                                src[:, k, n * 512:(n + 1) * 512],
                                start=(k == 0), stop=(k == 1))
                        nc.vector.tensor_scalar(
                            out=dst[:, m, n * 512:(n + 1) * 512], in0=ps[:, :],
                            scalar1=bf_sb[:, m:m + 1], scalar2=0.0, op0=OP.add, op1=OP.max)

            if int(os.environ.get("KBISECT", "9")) <= 2:
                _finish_early(faT[:, 0, :])
                continue

            # ---------------- attention dir 1: ET chunks [j, i] ----------------
            # consumers: s1 (ones-matmul, rowsum over j) and betaT_un (b_e as lhsT)
            R1 = acts.tile([128, L], F32, name="R1", tag="R1")
            R2 = acts.tile([128, L], F32, name="R2", tag="R2")
            betaT = acts.tile([128, 2, L], F32R, name="betaT", tag="betaT")
            alphaT = acts.tile([128, 2, L], F32R, name="alphaT", tag="alphaT")

            for direction in range(2):
                # direction 0: chunks over j (attT), exp bias bm, consumers s1/beta
                # direction 1: chunks over i (att), exp bias am, consumers s2/alpha
                if direction == 0:
                    lhsTsrc, rhssrc, biascols = fbT, faT, bmb_sb
                    attend_lhs, Rdst, outT = be, R1, betaT
                else:
                    lhsTsrc, rhssrc, biascols = faT, fbT, amb_sb
                    attend_lhs, Rdst, outT = ae, R2, alphaT

                sps = [pp.tile([128, 512], F32, name=f"sps{direction}{n}", tag="ps")
                       for n in range(2)]
                bps = [[pp.tile([128, 512], F32, name=f"bps{direction}{m}{n}", tag="ps")
                        for n in range(2)] for m in range(2)]
                for j in range(8):
                    et = ech.tile([128, L], F32R, name="et", tag="et")
                    for n in range(2):
                        ps = pp.tile([128, 512], F32, name="ps", tag="ps")
                        for k in range(2):
                            nc.tensor.matmul(
                                ps[:, :], lhsTsrc[:, k, j * 128:(j + 1) * 128],
                                rhssrc[:, k, n * 512:(n + 1) * 512],
                                start=(k == 0), stop=(k == 1))
                        nc.scalar.activation(
                            out=et[:, n * 512:(n + 1) * 512], in_=ps[:, :], func=AF.Exp,
                            bias=biascols[:, j:j + 1], scale=1.0)
                    for n in range(2):
                        nc.tensor.matmul(
                            sps[n][:, :], ones_sb[:, :], et[:, n * 512:(n + 1) * 512],
                            start=(j == 0), stop=(j == 7))
                    for m in range(2):
                        for n in range(2):
                            nc.tensor.matmul(
                                bps[m][n][:, :], attend_lhs[:, j, m * 128:(m + 1) * 128],
                                et[:, n * 512:(n + 1) * 512],
                                start=(j == 0), stop=(j == 7))
                for n in range(2):
                    nc.vector.tensor_scalar_add(
                        out=Rdst[:, n * 512:(n + 1) * 512], in0=sps[n][:, :], scalar1=1e-8)
                    nc.vector.reciprocal(
                        out=Rdst[:, n * 512:(n + 1) * 512], in_=Rdst[:, n * 512:(n + 1) * 512])
                for m in range(2):
                    for n in range(2):
                        nc.vector.tensor_mul(
                            out=outT[:, m, n * 512:(n + 1) * 512], in0=bps[m][n][:, :],
                            in1=Rdst[:, n * 512:(n + 1) * 512])

            if int(os.environ.get("KBISECT", "9")) <= 3:
                _finish_early(betaT[:, 0, :])
                continue

            # ---------------- G + mask + reduce ----------------
            for side in range(2):
                topT, lowT, M_sb = ((aeT, betaT, AM_sb) if side == 0
                                    else (beT, alphaT, BM_sb))
                v = acts.tile([128, 2, L], F32, name=f"v{side}", tag=f"v{side}")
                for m in range(2):
                    for n in range(2):
                        ps = pp.tile([128, 512], F32, name="ps", tag="ps")
                        for c in range(4):
                            src = topT if c < 2 else lowT
                            nc.tensor.matmul(
                                ps[:, :], wg_sb[:, c, m * 128:(m + 1) * 128],
                                src[:, c % 2, n * 512:(n + 1) * 512],
                                start=(c == 0), stop=(c == 3))
                        nc.scalar.activation(
                            out=v[:, m, n * 512:(n + 1) * 512], in_=ps[:, :], func=AF.Relu,
                            bias=bg_sb[:, m:m + 1], scale=1.0)
                    nc.vector.tensor_mul(out=v[:, m, :], in0=v[:, m, :], in1=M_sb[:, :])
                    nc.vector.reduce_sum(
                        out=res[:, 2 * side + m:2 * side + m + 1], in_=v[:, m, :], axis=AX)
                    nc.vector.reduce_max(
                        out=res[:, 4 + 2 * side + m:4 + 2 * side + m + 1],
                        in_=v[:, m, :], axis=AX)
            nc.gpsimd.dma_start(out=out[it], in_=res[:, :])
    nc.compile()
    return nc


_NC_CACHE = None
LAST_RESULTS = None


def _get_nc():
    global _NC_CACHE
    if _NC_CACHE is None:
        _NC_CACHE = _build()
    return _NC_CACHE


def kernel(a_embeds, b_embeds, a_mask, b_mask, W_proj, b_proj, W_F, b_F, W_G, b_G):
    global LAST_RESULTS
    a_embeds = np.asarray(a_embeds, np.float32)
    b_embeds = np.asarray(b_embeds, np.float32)
    amf = np.asarray(a_mask).astype(np.float32)
    bmf = np.asarray(b_mask).astype(np.float32)

    # xT with ones row for the bias; zero padding to 304 rows
    def xt(x):
        o = np.zeros((B, DP, L), np.float32)
        o[:, :D] = x.transpose(0, 2, 1)
        o[:, D] = 1.0
        return o

    xa = _round_fp32r(xt(a_embeds))
    xb = _round_fp32r(xt(b_embeds))
    wp = np.zeros((DP, H), np.float32)
    wp[:D] = np.asarray(W_proj, np.float32)
    wp[D] = np.asarray(b_proj, np.float32)
    wp = _round_fp32r(wp)
    wf = _round_fp32r(np.asarray(W_F, np.float32))
    wg = _round_fp32r(np.asarray(W_G, np.float32))
    bfc = np.ascontiguousarray(np.asarray(b_F, np.float32).reshape(2, 128).T)
    bgc = np.ascontiguousarray(np.asarray(b_G, np.float32).reshape(2, 128).T)
    # exp bias: 0 where mask==1, -100 where mask==0; per chunk column [128, 8]
    amb = np.ascontiguousarray(
        (amf.reshape(B, 8, 128).transpose(0, 2, 1) - 1.0) * (-MASK_BIAS))
    bmb = np.ascontiguousarray(
        (bmf.reshape(B, 8, 128).transpose(0, 2, 1) - 1.0) * (-MASK_BIAS))

    in_maps = []
    for c in range(NCORES):
        s = slice(c * IPC, (c + 1) * IPC)
        in_maps.append({
            "xa": np.ascontiguousarray(xa[s]),
            "xb": np.ascontiguousarray(xb[s]),
            "wp": wp, "wf": wf, "wg": wg, "bfc": bfc, "bgc": bgc,
            "amb": np.ascontiguousarray(amb[s]),
            "bmb": np.ascontiguousarray(bmb[s]),
            "onesd": np.ones((128, 128), np.float32),
            "amf": np.ascontiguousarray(amf[s]),
            "bmf": np.ascontiguousarray(bmf[s]),
        })

    nc = _get_nc()
    LAST_RESULTS = run_bass_kernel_spmd(nc, in_maps, core_ids=list(range(NCORES)))
    outs = np.concatenate([r["out"] for r in LAST_RESULTS.results], axis=0)
    return np.ascontiguousarray(outs.transpose(0, 2, 1).reshape(B, 4 * H))
